# revision 15
# baseline (speedup 1.0000x reference)
"""Trainium2 Bass kernel for nn_GATrBlock_61564061221554 (GATr block), v2.

kernel(**inputs) takes FULL inputs, returns FULL output [2, 2048, 64, 16].
Sharding: 8 cores = (batch b in 0..1) x (query chunk m in 0..3); each core
computes 512 query tokens against all 2048 keys. Token axis host-reordered to
[my 512 | rest].

v2 strategy vs v1:
  - all matmuls / SBUF tensors bf16 (fp32 matmul is a 2-pass op on trn2);
    x shipped to HBM as bf16 (half the DMA).
  - w2 folded into w1 for the value path (equivariant maps compose);
    w3 folded into the four bilinear-input linears.
  - key-side rstd folded into the V2 PSUM->SBUF copy (tensor_scalar with a
    per-partition scalar AP).
  - all transcendentals from one activation-table set (natural_log_exp):
    rstd = exp(-0.5 ln(ms+eps)), 1/z = exp(-ln z), gelu via exp/ln-composed
    tanh approximation. One ACT_TABLE_LOAD for the whole kernel.
  - attention: scores streamed per key-block, exp fused w/ scale on Scalar,
    AV with 8 concurrent PSUM accumulators, all bf16 N=512 streams.
  - geometric bilinear: quad outputs copied once per token-block into a
    type-major SBUF tile; lattice product instructions batch all 4 token
    blocks via 4-free-dim APs; reduction by in-place bf16 tree-adds.
  - output written feature-major; host transposes for free.
"""
import os
import sys
import numpy as np

for _p in ("/opt/trn_rl_repo",):
    if os.path.isdir(_p) and _p not in sys.path:
        sys.path.append(_p)

try:
    import ml_dtypes
    BF16 = ml_dtypes.bfloat16
except Exception:  # pragma: no cover
    BF16 = np.float32

# ---------------------------------------------------------------------------
# Host algebra tables
# ---------------------------------------------------------------------------
MASKS = sorted(range(16), key=lambda m: (bin(m).count("1"), m))
IDX = {m: i for i, m in enumerate(MASKS)}


def _popc(x):
    return bin(x).count("1")


def _B2(a, b):
    t, n = 0, a >> 1
    while n:
        t += _popc(n & b)
        n >>= 1
    return t & 1


def _chi(C, k):
    return -1.0 if (_popc(k & C) & 1) else 1.0


def _host_tables():
    Gm = np.zeros((16, 16, 16), np.float64)
    Om = np.zeros((16, 16, 16), np.float64)
    for a in range(16):
        for b in range(16):
            c = a ^ b
            s = -1.0 if _B2(a, b) else 1.0
            if not (a & b & 1):
                Gm[c, a, b] = s
            if a & b == 0:
                Om[c, a, b] = s
    D = np.zeros((16, 16), np.float64)
    U = np.zeros((16, 16), np.float64)
    for a in range(16):
        c = 15 ^ a
        D[c, a] = -1.0 if _B2(a, c) else 1.0
        U[a, c] = -1.0 if _B2(c, a) else 1.0
    Jm = np.einsum("ai,ijk,jb,kc->abc", U, Om, D, D)

    s1G = np.array([(-1.0) ** _B2(j, j) for j in range(16)])
    scB = np.array([(-1.0) ** _B2(i, i) for i in range(16)])
    T_of = []
    for i in range(16):
        T = 0
        for p in range(4):
            if _popc(i & ((1 << p) - 1)) & 1:
                T |= 1 << p
        T_of.append(T)
    for i in range(16):
        for j in range(16):
            k = j ^ i
            v = Gm[i, j, k]
            if j & k & 1:
                assert v == 0
            else:
                assert v == s1G[j] * _chi(T_of[i], k) * scB[i]

    sjJ = np.array([Jm[0, j, j ^ 15] for j in range(16)])
    U_of, cJ = [], []
    for i in range(16):
        it = 15 ^ i
        vals = {}
        for j in range(16):
            k = j ^ it
            if (j | k) == 15:
                vals[k] = Jm[i, j, k] / sjJ[j]
        fit = None
        ks = sorted(vals)
        for Uc in range(16):
            c0 = vals[ks[0]] * _chi(Uc, ks[0])
            if all(abs(vals[k] - c0 * _chi(Uc, k)) < 1e-9 for k in ks):
                fit = (Uc, c0)
                break
        assert fit is not None, i
        U_of.append(fit[0])
        cJ.append(fit[1])
    return dict(Gm=Gm, Jm=Jm, s1G=s1G, scB=scB, T_of=T_of, sjJ=sjJ,
                U_of=U_of, cJ=np.array(cJ))


TAB = _host_tables()


def _wblock(w, scale_out=None):
    """w: [O, 64, 9] -> [8, 128, 2*O] lhsT blocks; pair p = masks (2p, 2p+1).
    K rows: [x_partner(64); x_e0(64)]; M cols: [y_partner(O); y_e0(O)]."""
    O = w.shape[0]
    out = np.zeros((8, 128, 2 * O), np.float32)
    for p in range(8):
        mp = 2 * p
        g = _popc(mp)
        sp = 1.0 if scale_out is None else scale_out[mp]
        se = 1.0 if scale_out is None else scale_out[mp + 1]
        out[p, 0:64, 0:O] = w[:, :, g].T * sp
        out[p, 64:128, O:2 * O] = w[:, :, g + 1].T * se
        out[p, 0:64, O:2 * O] = w[:, :, 5 + g].T * se
    return out


def _w4block(w4):
    out = np.zeros((8, 128, 128), np.float32)
    for p in range(8):
        mp = 2 * p
        g = _popc(mp)
        blk = np.zeros((128, 128), np.float32)
        blk[0:64, 0:64] = w4[:, :, g].T
        blk[64:128, 64:128] = w4[:, :, g + 1].T
        blk[0:64, 64:128] = w4[:, :, 5 + g].T
        for side in (0, 1):
            mm = mp + side
            gp_c = _chi(TAB["T_of"][mm], mm) * TAB["scB"][mm]
            jn_c = _chi(TAB["U_of"][mm], 15 ^ mm) * TAB["cJ"][mm]
            blk[side * 64:side * 64 + 32, :] *= gp_c
            blk[side * 64 + 32:side * 64 + 64, :] *= jn_c
        out[p] = blk
    return out


# ---------------------------------------------------------------------------
# Bilinear op plan (verified host-side)
# ---------------------------------------------------------------------------
def _lattice_ops(i, table):
    if table == "gp":
        xor = i
        C = TAB["T_of"][i]
        fixed = {} if (i & 1) else {0: 0}
    else:
        xor = 15 ^ i
        C = TAB["U_of"][i]
        fixed = {b: 1 for b in range(4) if (i >> b) & 1}
    Rbits = [b for b in range(4) if b not in fixed]
    j_base = sum(v << b for b, v in fixed.items())
    RC = [b for b in Rbits if (C >> b) & 1]

    def mkop(sign, extra):
        jb = j_base | sum(v << b for b, v in extra.items())
        rb = [b for b in Rbits if b not in extra]
        rc = [b for b in rb if (C >> b) & 1]
        p_fixed = _popc(jb & C) & 1
        want = (0 if sign > 0 else 1) ^ p_fixed
        if not rc:
            if want:
                return None
            vecs, off = [[(b, +1)] for b in rb], jb
        else:
            piv = rc[0]
            off = jb | ((1 << piv) if want else 0)
            vecs = []
            for b in rb:
                if b == piv:
                    continue
                if b in rc:
                    vecs.append([(b, +1), (piv, +1 if want == 0 else -1)])
                else:
                    vecs.append([(b, +1)])
        dims = []
        for vec in vecs:
            vj = sum(d * (1 << b) for b, d in vec)
            vk = sum(d * (-(1 << b) if (xor >> b) & 1 else (1 << b))
                     for b, d in vec)
            dims.append((vj, vk, 2))
        merged = []
        for vj, vk, cnt in dims:
            if merged and merged[-1][0] * merged[-1][2] == vj \
                    and merged[-1][1] * merged[-1][2] == vk:
                pj, pk, pc = merged[-1]
                merged[-1] = (pj, pk, pc * 2)
            else:
                merged.append((vj, vk, cnt))
        return dict(j0=off, k0=off ^ xor, dims=merged, sign=sign)

    ops = []
    if len(RC) <= 2:
        for s in (+1, -1):
            op = mkop(s, {})
            if op is not None:
                ops.append(op)
    else:
        hb = RC[-1]
        for hv in (0, 1):
            for s in (+1, -1):
                op = mkop(s, {hb: hv})
                if op is not None:
                    ops.append(op)
    # cap: at most 2 lattice dims per op ([tb, d1, d2, ch] = 4 free dims)
    capped = []
    stack = list(ops)
    while stack:
        o = stack.pop(0)
        if len(o["dims"]) <= 2:
            capped.append(o)
            continue
        vj, vk, c = o["dims"][0]
        for s in range(c):
            stack.append(dict(j0=o["j0"] + vj * s, k0=o["k0"] + vk * s,
                              dims=list(o["dims"][1:]), sign=o["sign"]))
    ops = capped
    n_total = 1 << len(Rbits)

    def opn(o):
        n = 1
        for _, _, c in o["dims"]:
            n *= c
        return n

    assert sum(opn(o) for o in ops) == n_total
    return ops, n_total


BIL_PLAN = {(i, t): _lattice_ops(i, t)
            for i in range(16) for t in ("gp", "jn")}


def _verify_bilinear_plan():
    rng = np.random.default_rng(0)
    l = rng.standard_normal((16, 3))
    r = rng.standard_normal((16, 3))
    for table, tabm, sfold in (("gp", TAB["Gm"], TAB["s1G"]),
                               ("jn", TAB["Jm"], TAB["sjJ"])):
        lf = l * sfold[:, None]
        for i in range(16):
            want = np.einsum("jk,jc,kc->c", tabm[i], l, r)
            if table == "gp":
                c_i = _chi(TAB["T_of"][i], i) * TAB["scB"][i]
            else:
                c_i = _chi(TAB["U_of"][i], 15 ^ i) * TAB["cJ"][i]
            ops, _ = BIL_PLAN[(i, table)]
            got = np.zeros(3)
            import itertools
            for op in ops:
                ranges = [range(c) for _, _, c in op["dims"]]
                for sel in itertools.product(*ranges):
                    j, k = op["j0"], op["k0"]
                    for (vj, vk, _c), s in zip(op["dims"], sel):
                        j += vj * s
                        k += vk * s
                    got += op["sign"] * lf[j] * r[k]
            assert np.allclose(got * c_i, want), (table, i)


_verify_bilinear_plan()

# Regions: group (i, table) outputs by n_total; products [O, n, 32] per tb.
_PAIRS = [(i, t) for i in range(16) for t in ("gp", "jn")]
_REGS = {}
for (i, t) in _PAIRS:
    n = BIL_PLAN[(i, t)][1]
    _REGS.setdefault(n, []).append((i, t))
_REG_BASE = {}
_off = 0
for n in sorted(_REGS, reverse=True):
    if n == 1:
        continue
    _REG_BASE[n] = _off
    _off += len(_REGS[n]) * n * 32
PROD = _off


def _hoff(i, table):
    return i * 64 + (0 if table == "gp" else 32)


def _ap_runs(vals):
    """vals: list of (idx, value) sorted by idx. Return maximal runs
    (idx0, val0, stride, count) with consecutive idx and constant stride."""
    runs = []
    k = 0
    while k < len(vals):
        i0, v0 = vals[k]
        cnt = 1
        if k + 1 < len(vals) and vals[k + 1][0] == i0 + 1:
            stride = vals[k + 1][1] - v0
            while (k + cnt < len(vals)
                   and vals[k + cnt][0] == i0 + cnt
                   and vals[k + cnt][1] == v0 + stride * cnt):
                cnt += 1
        else:
            stride = 64
        runs.append((i0, v0, stride if cnt > 1 else 64, cnt))
        k += cnt
    return runs


# ---------------------------------------------------------------------------
# Device program
# ---------------------------------------------------------------------------
NCORES = 8
S = 2048
SQ = 512
H = 64
NT = S // 128
NTQ = SQ // 128

_PROG = None


def _build_program():
    import concourse.bass as bass  # noqa
    import concourse.bacc as bacc
    import concourse.tile as tile
    from concourse import mybir
    from concourse.ap import AP

    f32 = mybir.dt.float32
    bf16 = mybir.dt.bfloat16
    AO = mybir.AluOpType
    AF = mybir.ActivationFunctionType

    import concourse.bacc as _bacc_mod
    if not getattr(_bacc_mod, "_act_tables_filtered", False):
        _orig_gat = _bacc_mod.get_activation_tables

        def _only_nle(arch):
            t = _orig_gat(arch)
            keep = {k: v for k, v in t.items()
                    if k == "natural_log_exp_and_others"}
            return keep or t

        _bacc_mod.get_activation_tables = _only_nle
        _bacc_mod._act_tables_filtered = True

    nc = bacc.Bacc()
    xf_d = nc.declare_dram_parameter("xf", [8, 128, S], bf16, isOutput=False)
    wp_d = nc.declare_dram_parameter("wpack", [128, 4736], bf16,
                                     isOutput=False)
    out_d = nc.declare_dram_parameter("out", [8, 128, SQ], f32, isOutput=True)

    SC = float(1.0 / np.sqrt(8.0 * H))
    GC = float(np.sqrt(2.0 / np.pi))
    LNH2 = float(0.5 * np.log(H))
    EPSH = float(1e-6 * H)

    with tile.TileContext(nc) as tc:
      with tc.tile_pool(name="persist", bufs=1) as pp:
        wtall = pp.tile([128, 4736], bf16, tag="wtall")
        wt1q = wtall[:, 0:512]
        wt21 = wtall[:, 512:1536]
        wtq = wtall[:, 1536:3584]
        wt4 = wtall[:, 3584:4608]
        idt = wtall[:, 4608:4736]
        ones_b = pp.tile([128, 1], bf16, tag="ones_b")
        ones64 = pp.tile([64, 1], bf16, tag="ones64")
        onesrow = pp.tile([1, 128], bf16, tag="onesrow")
        one11 = pp.tile([1, 1], bf16, tag="one11")
        epsb = pp.tile([1, 1], f32, tag="epsb")
        lnhb = pp.tile([1, 1], f32, tag="lnhb")
        rstd_tm = pp.tile([128, NT], f32, tag="rstd_tm")
        rbs = pp.tile([128, S], bf16, tag="rbs")
        zbs = pp.tile([128, 512], bf16, tag="zbs")
        xb1f = pp.tile([128, 8 * 512], bf16, tag="xb1f")
        xn2 = pp.tile([128, 8 * 512], bf16, tag="xn2")
        r2s = pp.tile([128, 512], bf16, tag="r2s")
        outf = pp.tile([128, 8 * 512], f32, tag="outf")
        lnm = pp.tile([1, S], f32, tag="lnm")
        rstd_b = pp.tile([1, S], bf16, tag="rstd_b")

        nc.scalar.dma_start(wtall[:], wp_d[:])
        nc.vector.memset(ones_b[:], 1.0)
        nc.vector.memset(ones64[:], 1.0)
        nc.vector.memset(onesrow[:], 1.0)
        nc.vector.memset(one11[:], 1.0)
        nc.vector.memset(epsb[:], EPSH)
        nc.vector.memset(lnhb[:], LNH2)

        with tc.tile_pool(name="big1", bufs=1) as bp:
            xft = bp.tile([128, 8 * S], bf16, tag="xft")
            qt = bp.tile([128, 4 * S], bf16, tag="qt")
            vt = bp.tile([128, NT * 1024], bf16, tag="vt")
            et = bp.tile([128, NT * 512], bf16, tag="et")
            for p in range(8):
                eng = nc.sync if p % 2 == 0 else nc.scalar
                eng.dma_start(xft[:, p * S:(p + 1) * S], xf_d[p])

            # ---- norm1: ssum of squares over inner comps -----------------
            with tc.tile_pool(name="n1", bufs=2) as n1p, \
                 tc.tile_pool(name="ps1", bufs=2, space="PSUM") as ps1:
                ps_s2a = ps1.tile([1, 1024], f32, tag="s2")
                ps_s2b = ps1.tile([1, 1024], f32, tag="s2")
                for p in range(8):
                    sq = n1p.tile([64, S], bf16, tag=f"sq{p % 2}")
                    src = xft[0:64, p * S:(p + 1) * S]
                    if p % 2 == 0:
                        nc.vector.tensor_tensor(sq[:], src, src, AO.mult)
                    else:
                        nc.scalar.activation(sq[:], src, AF.Square)
                    for ck in range(4):
                        ps_s2 = ps_s2a if ck < 2 else ps_s2b
                        co = (ck % 2) * 512
                        nc.tensor.matmul(
                            ps_s2[:, co:co + 512], ones64[:],
                            sq[0:64, ck * 512:(ck + 1) * 512],
                            start=(p == 0), stop=(p == 7))
                for ci, ps_s2 in ((0, ps_s2a), (1, ps_s2b)):
                    nc.scalar.activation(lnm[:, ci * 1024:(ci + 1) * 1024],
                                         ps_s2[:], AF.Ln, bias=epsb[:])
                # rstd = exp(-0.5*(ln(ms+epsH) - ln(H))) = 1/sqrt(mean+eps)
                nc.scalar.activation(rstd_b[:], lnm[:], AF.Exp,
                                     bias=lnhb[:], scale=-0.5)

            with tc.tile_pool(name="n1b", bufs=1) as n1bp:
                with tc.tile_pool(name="ps1b", bufs=1, space="PSUM") as ps1b:
                    ps_rt = ps1b.tile([128, NT], f32, tag="rt")
                    for tb in range(NT):
                        nc.tensor.matmul(ps_rt[:, tb:tb + 1],
                                         rstd_b[:, tb * 128:(tb + 1) * 128],
                                         one11[:], start=True, stop=True)
                    nc.scalar.copy(rstd_tm[:], ps_rt[:])
                    for ci in range(2):
                        ps_rb = ps1b.tile([128, 1024], f32, tag="rb")
                        for cj in range(2):
                            co = ci * 1024 + cj * 512
                            nc.tensor.matmul(ps_rb[:, cj * 512:(cj + 1) * 512],
                                             onesrow[:],
                                             rstd_b[:, co:co + 512],
                                             start=True, stop=True)
                        nc.scalar.copy(
                            rbs[:, ci * 1024:(ci + 1) * 1024], ps_rb[:])

            # ---- q (feature-major, rstd-scaled) --------------------------
            with tc.tile_pool(name="psq", bufs=4, space="PSUM") as psq:
                for t in range(4):
                    for ci in range(2):
                        ps_q = psq.tile([128, 1024], f32, tag="psq")
                        for half in range(2):
                            p = 2 * t + half
                            for cj in range(2):
                                co = ci * 1024 + cj * 512
                                nc.tensor.matmul(
                                    ps_q[half * 64:(half + 1) * 64,
                                         cj * 512:(cj + 1) * 512],
                                    wt1q[:, p * 64:(p + 1) * 64],
                                    xft[:, p * S + co: p * S + co + 512],
                                    start=True, stop=True)
                        nc.vector.tensor_tensor(
                            qt[:, t * S + ci * 1024: t * S + (ci + 1) * 1024],
                            ps_q[:], rbs[:, ci * 1024:(ci + 1) * 1024],
                            AO.mult)

            # ---- V2 = (w2 o w1)(xn), token-major, rstd-scaled ------------
            with tc.tile_pool(name="psv", bufs=4, space="PSUM") as psv:
                for tb in range(NT):
                    ps_v = psv.tile([128, 1024], f32, tag="psv")
                    for p in range(8):
                        nc.tensor.matmul(
                            ps_v[:, p * 128:(p + 1) * 128],
                            xft[:, p * S + tb * 128: p * S + (tb + 1) * 128],
                            wt21[:, p * 128:(p + 1) * 128],
                            start=True, stop=True)
                    nc.vector.tensor_scalar(
                        vt[:, tb * 1024:(tb + 1) * 1024], ps_v[:],
                        rstd_tm[:, tb:tb + 1], None, AO.mult)

            # ---- attention scores + softmax numerator --------------------
            with tc.tile_pool(name="att", bufs=2) as atp, \
                 tc.tile_pool(name="pss", bufs=2, space="PSUM") as pss, \
                 tc.tile_pool(name="psz", bufs=1, space="PSUM") as psz:
                ps_z = psz.tile([1, 512], f32, tag="z")
                for kb in range(NT):
                    ps_s = pss.tile([128, 512], f32, tag="s")
                    for t in range(4):
                        nc.tensor.matmul(
                            ps_s[:],
                            qt[:, t * S + kb * 128: t * S + (kb + 1) * 128],
                            qt[:, t * S: t * S + 512],
                            start=(t == 0), stop=(t == 3))
                    nc.scalar.activation(et[:, kb * 512:(kb + 1) * 512],
                                         ps_s[:], AF.Exp, scale=SC)
                for kb in range(NT):
                    nc.tensor.matmul(ps_z[:], ones_b[:],
                                     et[:, kb * 512:(kb + 1) * 512],
                                     start=(kb == 0), stop=(kb == NT - 1))
                zln = atp.tile([1, 512], f32, tag="zln")
                zrow = atp.tile([1, 512], bf16, tag="zrow")
                nc.scalar.activation(zln[:], ps_z[:], AF.Ln)
                nc.scalar.activation(zrow[:], zln[:], AF.Exp, scale=-1.0)
                ps_zb = pss.tile([128, 512], f32, tag="s")
                nc.tensor.matmul(ps_zb[:], onesrow[:], zrow[:],
                                 start=True, stop=True)
                nc.scalar.copy(zbs[:], ps_zb[:])

            # ---- AV (8 concurrent PSUM accumulators) + xb1 ---------------
            with tc.tile_pool(name="av", bufs=2) as avp, \
                 tc.tile_pool(name="psav", bufs=8, space="PSUM") as psav:
                ps_list = []
                for vb in range(8):
                    ps_av = psav.tile([128, 512], f32, tag="av")
                    for kb in range(NT):
                        nc.tensor.matmul(
                            ps_av[:],
                            vt[:, kb * 1024 + vb * 128: kb * 1024 + (vb + 1) * 128],
                            et[:, kb * 512:(kb + 1) * 512],
                            start=(kb == 0), stop=(kb == NT - 1))
                    ps_list.append((vb, ps_av))
                for vb, ps_av in ps_list:
                    attv = avp.tile([128, 512], bf16, tag=f"attv{vb % 2}")
                    nc.vector.tensor_tensor(attv[:], ps_av[:], zbs[:], AO.mult)
                    nc.vector.tensor_tensor(
                        xb1f[:, vb * 512:(vb + 1) * 512], attv[:],
                        xft[:, vb * S: vb * S + 512], AO.add)

        # ---- norm2 + xn2 (big1 closed; xft freed) ------------------------
        with tc.tile_pool(name="n2", bufs=2) as n2p, \
             tc.tile_pool(name="ps2", bufs=2, space="PSUM") as ps2:
            ps_m2 = ps2.tile([1, 512], f32, tag="m2")
            for p in range(8):
                sq2 = n2p.tile([64, 512], bf16, tag=f"sq2{p % 2}")
                src = xb1f[0:64, p * 512:(p + 1) * 512]
                if p % 2 == 0:
                    nc.vector.tensor_tensor(sq2[:], src, src, AO.mult)
                else:
                    nc.scalar.activation(sq2[:], src, AF.Square)
                nc.tensor.matmul(ps_m2[:], ones64[:], sq2[:],
                                 start=(p == 0), stop=(p == 7))
            ln2 = n2p.tile([1, 512], f32, tag="ln2")
            rstd2 = n2p.tile([1, 512], bf16, tag="rstd2")
            nc.scalar.activation(ln2[:], ps_m2[:], AF.Ln, bias=epsb[:])
            nc.scalar.activation(rstd2[:], ln2[:], AF.Exp,
                                 bias=lnhb[:], scale=-0.5)
            ps_r2 = ps2.tile([128, 512], f32, tag="r2")
            nc.tensor.matmul(ps_r2[:], onesrow[:], rstd2[:],
                             start=True, stop=True)
            nc.scalar.copy(r2s[:], ps_r2[:])
            for p in range(8):
                nc.vector.tensor_tensor(xn2[:, p * 512:(p + 1) * 512],
                                        xb1f[:, p * 512:(p + 1) * 512],
                                        r2s[:], AO.mult)

        # ---- bilinear branch ---------------------------------------------
        with tc.tile_pool(name="bil", bufs=1) as bl, \
             tc.tile_pool(name="psQ", bufs=1, space="PSUM") as psQ, \
             tc.tile_pool(name="psH", bufs=2, space="PSUM") as psH, \
             tc.tile_pool(name="psO", bufs=2, space="PSUM") as psO:
            quadT = bl.tile([128, 4 * NTQ * 512], bf16, tag="quadT")
            rng_t = bl.tile([128, NTQ * 512], bf16, tag="rng")
            jng_t = bl.tile([128, NTQ * 512], bf16, tag="jng")
            prod = bl.tile([128, NTQ * PROD], bf16, tag="prod")
            hraw = bl.tile([128, NTQ * 1024], bf16, tag="hraw")
            htm = bl.tile([128, NTQ * 1024], bf16, tag="htm")
            hf = bl.tile([128, NTQ * 1024], bf16, tag="hf")
            gwt = bl.tile([128, 6 * 256], bf16, tag="gw")

            def _pd(t):
                return list(list(t[:].ap)[0])

            for tb in range(NTQ):
                ps_q4 = psQ.tile([128, 2048], f32, tag="q4")
                for p in range(8):
                    nc.tensor.matmul(
                        ps_q4[:, p * 256:(p + 1) * 256],
                        xn2[:, p * 512 + tb * 128: p * 512 + (tb + 1) * 128],
                        wtq[:, p * 256:(p + 1) * 256],
                        start=True, stop=True)
                pq = list(list(ps_q4[:].ap)[0])
                for t in range(4):
                    src = AP(ps_q4[:].tensor, ps_q4[:].offset + t * 64,
                             [pq, [256, 8], [32, 2], [1, 32]])
                    dst = AP(quadT[:].tensor,
                             quadT[:].offset + t * NTQ * 512 + tb * 512,
                             [_pd(quadT), [64, 8], [32, 2], [1, 32]])
                    nc.scalar.copy(dst, src)
            nc.vector.tensor_scalar_mul(
                rng_t[:], quadT[:, 1 * NTQ * 512: 2 * NTQ * 512], -1.0)
            nc.vector.tensor_scalar_mul(
                jng_t[:], quadT[:, 3 * NTQ * 512: 4 * NTQ * 512], -1.0)

            qv = quadT[:]
            lbase = {"gp": 0, "jn": 2 * NTQ * 512}
            rbase = {"gp": 1 * NTQ * 512, "jn": 3 * NTQ * 512}

            def lat_view(base_ap, base_off, vecs):
                dims = [list(list(base_ap.ap)[0]), [512, NTQ]] \
                    + [[v, c] for v, c in vecs] + [[1, 32]]
                return AP(base_ap.tensor, base_ap.offset + base_off, dims)

            # products, tb-batched, <=1 lattice dim per instr
            for n in sorted(_REGS, reverse=True):
                for oi, (i, table) in enumerate(_REGS[n]):
                    ops, n_tot = BIL_PLAN[(i, table)]
                    neg = rng_t[:] if table == "gp" else jng_t[:]
                    if n == 1:
                        op = ops[0]
                        in0 = lat_view(qv, lbase[table] + op["j0"] * 32, [])
                        srct = qv if op["sign"] > 0 else neg
                        boff = rbase[table] if op["sign"] > 0 else 0
                        in1 = lat_view(srct, boff + op["k0"] * 32, [])
                        outv = AP(hraw[:].tensor,
                                  hraw[:].offset + _hoff(i, table),
                                  [_pd(hraw), [1024, NTQ], [1, 32]])
                        nc.vector.tensor_tensor(outv, in0, in1, AO.mult)
                        continue
                    obase = _REG_BASE[n] + oi * n * 32
                    slot = 0
                    for op in ops:
                        dims = list(op["dims"])
                        if dims:
                            li = max(range(len(dims)),
                                     key=lambda d: dims[d][2])
                            last = dims.pop(li)
                            peel = dims
                        else:
                            last, peel = None, []
                        import itertools as _it
                        for sel in _it.product(*[range(c) for _, _, c in peel]):
                            j0, k0 = op["j0"], op["k0"]
                            for (vj, vk, _c), sv in zip(peel, sel):
                                j0 += vj * sv
                                k0 += vk * sv
                            vecs0 = [(last[0] * 32, last[2])] if last else []
                            vecs1 = [(last[1] * 32, last[2])] if last else []
                            cl = last[2] if last else 1
                            in0 = lat_view(qv, lbase[table] + j0 * 32, vecs0)
                            srct = qv if op["sign"] > 0 else neg
                            boff = rbase[table] if op["sign"] > 0 else 0
                            in1 = lat_view(srct, boff + k0 * 32, vecs1)
                            outv = AP(prod[:].tensor,
                                      prod[:].offset + obase + slot * 32,
                                      [_pd(prod), [PROD, NTQ], [32, cl],
                                       [1, 32]])
                            nc.vector.tensor_tensor(outv, in0, in1, AO.mult)
                            slot += cl
                    assert slot == n_tot

            # packed-front in-place tree reduction (3-free-dim APs)
            for n in sorted(_REGS, reverse=True):
                if n == 1:
                    continue
                O = len(_REGS[n])
                base = _REG_BASE[n]
                s = n
                while s > 2:
                    h = s // 2
                    in0 = AP(prod[:].tensor, prod[:].offset + base,
                             [_pd(prod), [PROD, NTQ], [64, O * h], [1, 32]])
                    in1 = AP(prod[:].tensor, prod[:].offset + base + 32,
                             [_pd(prod), [PROD, NTQ], [64, O * h], [1, 32]])
                    outv = AP(prod[:].tensor, prod[:].offset + base,
                              [_pd(prod), [PROD, NTQ], [32, O * h], [1, 32]])
                    nc.vector.tensor_tensor(outv, in0, in1, AO.add)
                    s = h
                offs = [(k, _hoff(i, t))
                        for k, (i, t) in enumerate(_REGS[n])]
                for (k0, v0, stride, cnt) in _ap_runs(offs):
                    in0 = AP(prod[:].tensor,
                             prod[:].offset + base + k0 * 64,
                             [_pd(prod), [PROD, NTQ], [64, cnt], [1, 32]])
                    in1 = AP(prod[:].tensor,
                             prod[:].offset + base + k0 * 64 + 32,
                             [_pd(prod), [PROD, NTQ], [64, cnt], [1, 32]])
                    outv = AP(hraw[:].tensor, hraw[:].offset + v0,
                              [_pd(hraw), [1024, NTQ], [stride, cnt],
                               [1, 32]])
                    nc.vector.tensor_tensor(outv, in0, in1, AO.add)

            # ---- gate: gelu(x) = x*(1 - 1/(1+exp(2*GC*u))) ---------------
            def gv(col):
                return gwt[:, col * 256:(col + 1) * 256]

            xg_in = AP(hraw[:].tensor, hraw[:].offset,
                       [_pd(hraw), [1024, NTQ], [1, 64]])
            xg_out = AP(gwt[:].tensor, gwt[:].offset,
                        [_pd(gwt), [64, 4], [1, 64]])
            nc.vector.tensor_scalar_mul(xg_out, xg_in, 1.0)
            nc.vector.tensor_tensor(gv(1), gv(0), gv(0), AO.mult)
            nc.vector.tensor_scalar(gv(2), gv(1), 0.044715, 1.0,
                                    AO.mult, AO.add)
            nc.vector.tensor_tensor(gv(3), gv(2), gv(0), AO.mult)
            t_t = bl.tile([128, 256], f32, tag="gt")
            v_t = bl.tile([128, 256], f32, tag="gv2")
            gc_t = bl.tile([128, 256], bf16, tag="gc")
            nc.scalar.activation(t_t[:], gv(3), AF.Exp, scale=2.0 * GC)
            nc.scalar.activation(v_t[:], t_t[:], AF.Ln, bias=1.0)
            nc.scalar.activation(gc_t[:], v_t[:], AF.Exp, scale=-1.0)
            nc.vector.tensor_tensor(gv(4), gv(0), gc_t[:], AO.mult)
            nc.vector.tensor_tensor(gv(5), gv(0), gv(4), AO.subtract)
            for tb in range(NTQ):
                h_in = AP(hraw[:].tensor, hraw[:].offset + tb * 1024,
                          [_pd(hraw), [64, 16], [1, 64]])
                g_in = AP(gwt[:].tensor, gwt[:].offset + 5 * 256 + tb * 64,
                          [_pd(gwt), [0, 16], [1, 64]])
                h_out = AP(htm[:].tensor, htm[:].offset + tb * 1024,
                           [_pd(htm), [64, 16], [1, 64]])
                nc.vector.tensor_tensor(h_out, h_in, g_in, AO.mult)

            # ---- transpose htm -> hf [fb(8) x 512 tok] -------------------
            for tb in range(NTQ):
                for hb in range(2):
                    ps_h = psH.tile([128, 512], bf16, tag="h")
                    for q in range(4):
                        fb = hb * 4 + q
                        nc.tensor.transpose(
                            ps_h[:, q * 128:(q + 1) * 128],
                            htm[:, tb * 1024 + fb * 128:
                                tb * 1024 + (fb + 1) * 128],
                            idt[:])
                    dst = AP(hf[:].tensor,
                             hf[:].offset + hb * 4 * 512 + tb * 128,
                             [_pd(hf), [512, 4], [1, 128]])
                    nc.scalar.copy(dst, ps_h[:])

            # ---- w4 (feature-major) + residual + out -------------------
            for p in range(8):
                ps_o = psO.tile([128, 512], f32, tag="o")
                nc.tensor.matmul(ps_o[:], wt4[:, p * 128:(p + 1) * 128],
                                 hf[:, p * 512:(p + 1) * 512],
                                 start=True, stop=True)
                nc.vector.tensor_tensor(outf[:, p * 512:(p + 1) * 512],
                                        ps_o[:],
                                        xb1f[:, p * 512:(p + 1) * 512],
                                        AO.add)
                nc.sync.dma_start(out_d[p], outf[:, p * 512:(p + 1) * 512])
    return nc


# ---------------------------------------------------------------------------
# Host wrapper
# ---------------------------------------------------------------------------
def _host_weights(inputs, ref_e0123_by_b):
    w1 = np.asarray(inputs["w1"], np.float32)
    w2 = np.asarray(inputs["w2"], np.float32)
    w3 = np.asarray(inputs["w3"], np.float32)
    wl = np.asarray(inputs["wl"], np.float32)
    wr = np.asarray(inputs["wr"], np.float32)
    wjl = np.asarray(inputs["wjl"], np.float32)
    wjr = np.asarray(inputs["wjr"], np.float32)
    w4 = np.asarray(inputs["w4"], np.float32)

    L1 = _wblock(w1)
    L2 = _wblock(w2)
    w21 = np.einsum("pkm,pmn->pkn", L1, L2)
    w1q = np.ascontiguousarray(L1[:, :, 0:64])
    L3 = _wblock(w3)
    Lr = np.einsum("pkm,pmn->pkn", L3, _wblock(wr))
    Ljr = np.einsum("pkm,pmn->pkn", L3, _wblock(wjr))
    Ll = np.einsum("pkm,pmn->pkn", L3, _wblock(wl, scale_out=TAB["s1G"]))
    w4b = _w4block(w4)
    out = []
    for b in range(len(ref_e0123_by_b)):
        Ljl = np.einsum(
            "pkm,pmn->pkn", L3,
            _wblock(wjl, scale_out=TAB["sjJ"] * ref_e0123_by_b[b]))
        wqb = np.zeros((8, 128, 256), np.float32)
        for p in range(8):
            wqb[p, :, 0:64] = Ll[p]
            wqb[p, :, 64:128] = Lr[p]
            wqb[p, :, 128:192] = Ljl[p]
            wqb[p, :, 192:256] = Ljr[p]
        wpack = np.zeros((128, 4736), np.float32)
        for p in range(8):
            wpack[:, p * 64:(p + 1) * 64] = w1q[p]
            wpack[:, 512 + p * 128:512 + (p + 1) * 128] = w21[p]
            wpack[:, 1536 + p * 256:1536 + (p + 1) * 256] = wqb[p]
            wpack[:, 3584 + p * 128:3584 + (p + 1) * 128] = w4b[p]
        wpack[:, 4608:4736] = np.eye(128, dtype=np.float32)
        out.append(dict(wpack=wpack.astype(BF16)))
    return out


def _host_prep(inputs):
    x = np.asarray(inputs["x"], np.float32)
    ref = np.asarray(inputs["ref"], np.float32)
    perm = np.array([IDX[m] for m in range(16)])
    wsets = _host_weights(inputs, [float(ref[b, 0, 0, 15]) for b in range(2)])

    in_maps = []
    for core in range(NCORES):
        b, m = divmod(core, 4)
        lo = m * SQ
        order = np.concatenate([np.arange(lo, lo + SQ),
                                np.arange(0, lo),
                                np.arange(lo + SQ, S)])
        xc = x[b][order][:, :, perm]            # [S, 64, 16 mask-major]
        xt = xc.transpose(2, 1, 0)              # [16, 64, S]
        xf8 = np.ascontiguousarray(xt.reshape(8, 128, S)).astype(BF16)
        in_maps.append({"xf": xf8, "wpack": wsets[b]["wpack"]})
    return in_maps


_LDW_PATCHED = False


def _patch_ldw_opt():
    global _LDW_PATCHED
    if _LDW_PATCHED:
        return
    from concourse import bass_utils as _bu
    _orig = _bu.run_command

    def _rc(cmd, *a, **kw):
        if isinstance(cmd, list):
            cmd = [c for c in cmd]
        return _orig(cmd, *a, **kw)

    _bu.run_command = _rc
    _LDW_PATCHED = True


def kernel(**inputs):
    global _PROG
    _patch_ldw_opt()
    from concourse.bass_utils import run_bass_kernel_spmd
    if _PROG is None:
        _PROG = _build_program()
        if not _PROG.is_finalized():
            _PROG.finalize()
    in_maps = _host_prep(inputs)
    res = run_bass_kernel_spmd(_PROG, in_maps, list(range(NCORES)),
                               trace=bool(os.environ.get("KTRACE")))
    kernel.last_results = res
    x = np.asarray(inputs["x"])
    out = np.zeros(x.shape, np.float32)
    perm = np.array([IDX[m] for m in range(16)])
    for core in range(NCORES):
        b, m = divmod(core, 4)
        o = np.asarray(res.results[core]["out"]).astype(np.float32)
        full = np.empty((SQ, H, 16), np.float32)
        for p in range(8):
            full[:, :, perm[2 * p]] = o[p, 0:64].T
            full[:, :, perm[2 * p + 1]] = o[p, 64:128].T
        out[b, m * SQ:(m + 1) * SQ] = full
    return out
